# revision 1
# baseline (speedup 1.0000x reference)
"""BatchMatchedMSELoss on 8 Trainium2 NeuronCores.

loss = mean(concat(row_min, col_min)) of the (B,B) pairwise-MSE matrix
  mse[i,j] = (||x_i||^2 + ||y_j||^2 - 2 x_i.y_j) / D,  B=8192, D=1024.

Sharding: input rows split across 8 cores (1024 rows each); every core
computes its (1024, 8192) tile of D*mse = sqx[i] + sqy[j] - 2*cross via
bf16 matmuls with fp32 PSUM accumulation. The host hands each core
contraction-major bf16 operands (pure layout/dtype prep — the TensorE
stream is then pure matmul) and the fp32 sq terms are folded in by the
DVE epilogue's scalar_tensor_tensor pass. Row mins leave the device complete;
column partial mins (full 128 partitions) are combined
on the host along with the final mean.
"""

import numpy as np
import ml_dtypes

import concourse.bass as bass
import concourse.tile as tile
import concourse.mybir as mybir
from concourse.bass import ts
from concourse.bass_utils import run_bass_kernel_spmd

FP32 = mybir.dt.float32
BF16 = mybir.dt.bfloat16
AL = mybir.AluOpType
AX = mybir.AxisListType

B = 8192          # batch (rows of input and target)
D = 1024          # feature dim (contraction)
NCORES = 8
RPC = B // NCORES  # rows per core = 1024
P = 128
MT = RPC // P      # 8 row tiles per core
DT = D // P        # 8 contraction tiles
CHUNK = 1024       # column chunk
NCH = B // CHUNK   # 8 chunks
HALF = 512         # max moving free dim per matmul / one PSUM bank


def _legalize_waits(nc, max_waits=1):
    """walrus codegen in this container rejects instructions carrying more
    than one sync-wait command. Split extra waits onto standalone
    EventSemaphore instructions (same engine, immediately before), which is
    exactly what engine.wait_ge() emits."""
    n = 0
    for f in nc.m.functions:
        for bb in f.blocks:
            insts = bb.instructions
            out = []
            for inst in insts:
                si = inst.sync_info
                if si is not None and si.on_wait and len(si.on_wait) > max_waits:
                    waits = list(si.on_wait)
                    extra, keep = waits[:-max_waits], waits[-max_waits:]
                    for w in extra:
                        n += 1
                        ev = mybir.InstEventSemaphore(
                            name=f"legwait-{n}-{inst.name}", ins=[], outs=[]
                        )
                        ev.engine = inst.engine
                        ev.sync_info = mybir.SyncInfo(on_wait=[w], on_update=[])
                        out.append(ev)
                    inst.sync_info = mybir.SyncInfo(
                        on_wait=keep, on_update=list(si.on_update)
                    )
                out.append(inst)
            bb.instructions = out
    return n


def _hoist_pe_waits(nc, dist=4):
    """Move every PE-stream wait onto a standalone EventSemaphore `dist` PE
    instructions earlier, so the wait check overlaps the previous matmul's
    streaming instead of stalling the next group's start. Safe: waits only
    get stricter when moved earlier, and all PE waits here depend on progress
    many more than `dist` PE slots back (psum bufs=8 rotation, yt bufs=2)."""
    n = 0
    for f in nc.m.functions:
        for bb in f.blocks:
            insts = list(bb.instructions)
            pe_pos = [
                i for i, inst in enumerate(insts)
                if inst.engine == mybir.EngineType.PE
            ]
            pos_rank = {p: r for r, p in enumerate(pe_pos)}
            inserts = {}
            for p in pe_pos:
                inst = insts[p]
                # only data-dep waits on matmul/ldweights may move; barrier
                # EventSemaphore waits must stay after PE's own barrier inc
                if getattr(inst, "opcode", "") not in ("Matmult", "Ldweights"):
                    continue
                si = inst.sync_info
                if si is None or not si.on_wait:
                    continue
                tgt = pe_pos[max(0, pos_rank[p] - dist)]
                for w in si.on_wait:
                    n += 1
                    ev = mybir.InstEventSemaphore(
                        name=f"hoist-{n}-{inst.name}", ins=[], outs=[]
                    )
                    ev.engine = mybir.EngineType.PE
                    ev.sync_info = mybir.SyncInfo(on_wait=[w], on_update=[])
                    inserts.setdefault(tgt, []).append(ev)
                inst.sync_info = mybir.SyncInfo(
                    on_wait=[], on_update=list(si.on_update)
                )
            if inserts:
                out = []
                for i, inst in enumerate(insts):
                    out.extend(inserts.get(i, ()))
                    out.append(inst)
                bb.instructions = out
    return n


def build_bass(legalize: bool = True) -> bass.Bass:
    nc = bass.Bass()
    # xt = bf16((-2 * X_shard).T) [D, RPC]; yt = bf16(Y.T) [D, B]
    xt = nc.dram_tensor("xt", [D, RPC], BF16, kind="ExternalInput")
    yt = nc.dram_tensor("yt", [D, B], BF16, kind="ExternalInput")
    # fp32 sq terms, added in the DVE epilogue (no tail matmul)
    sqx_d = nc.dram_tensor("sqx", [P, MT], FP32, kind="ExternalInput")
    sqyb_d = nc.dram_tensor("sqyb", [P, B], FP32, kind="ExternalInput")
    rowmin_d = nc.dram_tensor("rowmin", [P, MT * NCH * 2], FP32, kind="ExternalOutput")
    # column partial mins, full 128 partitions; host finishes the min
    colmin_d = nc.dram_tensor("colmin", [P, B], FP32, kind="ExternalOutput")

    with tile.TileContext(nc) as tc:
        with (
            tc.tile_pool(name="consts", bufs=1) as consts,
            tc.tile_pool(name="yt8", bufs=3) as ytp,
            tc.tile_pool(name="thinp", bufs=3) as thinp,
            tc.tile_pool(name="work", bufs=3) as work,
            tc.tile_pool(name="pmm", bufs=8, space=bass.MemorySpace.PSUM) as pmm,
        ):
            rowmin_ch = consts.tile([P, MT * NCH * 2], FP32)
            sqx = consts.tile([P, MT], FP32)
            nc.sync.dma_start(out=sqx[:], in_=sqx_d[:, :])
            XT = [
                consts.tile([P, RPC], BF16, tag=f"xt{d}", name=f"xt{d}")
                for d in range(DT)
            ]

            # ---- Phase A: load X^T (already bf16) ----
            for dt in range(DT):
                for hf in range(2):
                    nc.sync.dma_start(
                        out=XT[dt][:, ts(hf, RPC // 2)],
                        in_=xt[ts(dt, P), ts(hf, RPC // 2)],
                    )

            # ---- Phase B: stream column chunks of Y^T ----
            for ch in range(NCH):
                j0 = ch * CHUNK
                sqyb = thinp.tile([P, CHUNK], FP32, tag="sqyb")
                nc.sync.dma_start(out=sqyb[:], in_=sqyb_d[:, j0 : j0 + CHUNK])
                yts = []
                for dt in range(DT):
                    ytile = ytp.tile([P, CHUNK], BF16, tag=f"yt{dt}", name=f"yt{dt}")
                    for hf in range(2):
                        nc.sync.dma_start(
                            out=ytile[:, ts(hf, HALF)],
                            in_=yt[ts(dt, P), j0 + hf * HALF : j0 + (hf + 1) * HALF],
                        )
                    yts.append(ytile)

                colmin = work.tile([P, CHUNK], FP32, tag="colmin")
                for m in range(MT):
                    for h in range(2):
                        hs = slice(h * HALF, (h + 1) * HALF)
                        ps = pmm.tile([P, HALF], FP32, tag="ps")
                        for dt in range(DT):
                            nc.tensor.matmul(
                                ps[:],
                                XT[dt][:, ts(m, P)],
                                yts[dt][:, hs],
                                start=(dt == 0),
                                stop=(dt == DT - 1),
                            )
                        k = (m * NCH + ch) * 2 + h
                        mse = work.tile([P, HALF], FP32, tag="mse")
                        nc.vector.scalar_tensor_tensor(
                            mse[:], ps[:], sqx[:, m : m + 1], sqyb[:, hs],
                            op0=AL.add, op1=AL.add,
                        )
                        nc.vector.tensor_reduce(
                            out=rowmin_ch[:, k : k + 1], in_=mse[:],
                            axis=AX.X, op=AL.min,
                        )
                        if m == 0:
                            nc.vector.tensor_copy(colmin[:, hs], mse[:])
                        else:
                            nc.vector.tensor_tensor(
                                colmin[:, hs], colmin[:, hs], mse[:], AL.min
                            )

                nc.sync.dma_start(
                    out=colmin_d[:, j0 : j0 + CHUNK], in_=colmin[:, :]
                )

            nc.sync.dma_start(out=rowmin_d[:, :], in_=rowmin_ch[:, :])
    if legalize:
        _legalize_waits(nc)
    return nc


_NC_CACHE = None


def _get_nc():
    global _NC_CACHE
    if _NC_CACHE is None:
        _NC_CACHE = build_bass()
    return _NC_CACHE


def _hi_lo(v):
    hi = v.astype(ml_dtypes.bfloat16)
    lo = (v - hi.astype(np.float64)).astype(ml_dtypes.bfloat16)
    return hi, lo


def _prep_inputs(X, Y):
    """Host-side sharding/layout: contraction-major bf16 operands + packed
    sq rows."""
    yt = np.ascontiguousarray(Y.T.astype(ml_dtypes.bfloat16))
    sqy = (Y.astype(np.float64) ** 2).sum(axis=1).astype(np.float32)
    sqyb = np.ascontiguousarray(np.broadcast_to(sqy, (P, B)))

    in_maps = []
    for c in range(NCORES):
        Xs = X[c * RPC : (c + 1) * RPC]
        xt = np.ascontiguousarray((-2.0 * Xs).T.astype(ml_dtypes.bfloat16))
        sqx = (Xs.astype(np.float64) ** 2).sum(axis=1).astype(np.float32)
        sqx_pm = np.ascontiguousarray(sqx.reshape(MT, P).T)
        in_maps.append({"xt": xt, "yt": yt, "sqx": sqx_pm, "sqyb": sqyb})
    return in_maps


def kernel(input, target):
    X = np.ascontiguousarray(np.asarray(input, dtype=np.float32))
    Y = np.ascontiguousarray(np.asarray(target, dtype=np.float32))
    assert X.shape == (B, D) and Y.shape == (B, D)

    nc = _get_nc()
    in_maps = _prep_inputs(X, Y)
    try:
        res = run_bass_kernel_spmd(nc, in_maps, core_ids=list(range(NCORES))).results
    except Exception:
        # a prior process can leave a core wedged; one retry clears it
        res = run_bass_kernel_spmd(nc, in_maps, core_ids=list(range(NCORES))).results

    row_sum = np.float64(0.0)
    col_parts = []
    for r in res:
        rm = r["rowmin"].reshape(P, MT, NCH * 2).min(axis=2)
        row_sum += rm.astype(np.float64).sum()
        col_parts.append(r["colmin"].min(axis=0))
    col_min = np.min(np.stack(col_parts), axis=0).astype(np.float64)
    loss = (row_sum + col_min.sum()) / D / (2 * B)
    return np.asarray(loss, dtype=np.float32)



# revision 3
# speedup vs baseline: 1.9356x; 1.9356x over previous
"""BatchMatchedMSELoss on 8 Trainium2 NeuronCores.

loss = mean(concat(row_min, col_min)) of the (B,B) pairwise-MSE matrix
  mse[i,j] = (||x_i||^2 + ||y_j||^2 - 2 x_i.y_j) / D,  B=8192, D=1024.

Sharding: input rows split across 8 cores (1024 rows each); every core
computes its (1024, 8192) tile of the centered matrix
  cmse[i,j] = D*mse[i,j] - 2048 = (sqx_i-1024) + (sqy_j-1024) - 2 x_i.y_j
via fp8(e4m3) DoubleRow matmuls (K=256/instruction, 2x bf16 TensorE
throughput). Two of the 1024 contraction slots are donated to carry
-0.5*(sqy-1024) as an fp8 hi/lo pair (x-side slots = 1.0), so PSUM already
holds x.y_1022 - 0.5*sqy_c and no per-column vector add is needed later.
The epilogue splits across the remaining engines to hide behind the
matmul stream:
  * Act : evicts two PSUM banks per op as fp16(-2*psum + sqx_c[m])
          (scale/bias fused; per-partition bias AP) -> full cmse tile
  * DVE : two fast fp16 tensor_tensor(min) accumulations per tile
          (row accumulator per m, col-min per chunk) + one small
          tensor_reduce per m at the end
Host adds back the exact 2048 offset, finishes cross-core/partition
mins and the final mean in fp64. Simulated rel err vs fp32 reference:
2.6e-4 (tolerance 2e-2).
"""

import numpy as np
import ml_dtypes

import concourse.bass as bass
import concourse.tile as tile
import concourse.mybir as mybir
from concourse.bass import ts
from concourse.bass_utils import run_bass_kernel_spmd

FP32 = mybir.dt.float32
FP16 = mybir.dt.float16
FP8 = mybir.dt.float8e4
AL = mybir.AluOpType
AF = mybir.ActivationFunctionType
AX = mybir.AxisListType

B = 8192          # batch (rows of input and target)
D = 1024          # feature dim (contraction); last 2 slots carry sqy hi/lo
DF = D - 2        # real features used in the fp8 cross product
NCORES = 8
RPC = B // NCORES  # rows per core = 1024
P = 128
MT = RPC // P      # 8 row tiles per core
KG = 4             # DoubleRow k-groups (256 contraction rows each)
CHUNK = 1024       # column chunk = one PSUM double-bank eviction
NCH = B // CHUNK   # 8 chunks
HALF = 512         # max moving free dim per matmul / one PSUM bank

NP_FP8 = ml_dtypes.float8_e4m3


def _legalize_waits(nc, max_waits=1):
    """walrus codegen in this container rejects instructions carrying more
    than one sync-wait command. Split extra waits onto standalone
    EventSemaphore instructions (same engine, immediately before), which is
    exactly what engine.wait_ge() emits."""
    n = 0
    for f in nc.m.functions:
        for bb in f.blocks:
            insts = bb.instructions
            out = []
            for inst in insts:
                si = inst.sync_info
                if si is not None and si.on_wait and len(si.on_wait) > max_waits:
                    waits = list(si.on_wait)
                    extra, keep = waits[:-max_waits], waits[-max_waits:]
                    for w in extra:
                        n += 1
                        ev = mybir.InstEventSemaphore(
                            name=f"legwait-{n}-{inst.name}", ins=[], outs=[]
                        )
                        ev.engine = inst.engine
                        ev.sync_info = mybir.SyncInfo(on_wait=[w], on_update=[])
                        out.append(ev)
                    inst.sync_info = mybir.SyncInfo(
                        on_wait=keep, on_update=list(si.on_update)
                    )
                out.append(inst)
            bb.instructions = out
    return n


def build_bass(legalize: bool = True) -> bass.Bass:
    nc = bass.Bass()
    # fp8 operands, contraction-major, DoubleRow layout [128, 2, cols]:
    # element [p, s, c] holds contraction row k = kg*256 + s*128 + p.
    xt_d = [
        nc.dram_tensor(f"xt{kg}", [P, 2, RPC], FP8, kind="ExternalInput")
        for kg in range(KG)
    ]
    yt_d = [
        nc.dram_tensor(f"yt{kg}", [P, 2, B], FP8, kind="ExternalInput")
        for kg in range(KG)
    ]
    # centered fp32 row sq-norms: sqx[p, m] = |x_{m*128+p}|^2 - 1024
    sqx_d = nc.dram_tensor("sqx", [P, MT], FP32, kind="ExternalInput")
    # per-(p, m) row minima of the centered matrix
    rowmin_d = nc.dram_tensor("rowmin", [P, MT], FP32, kind="ExternalOutput")
    # column partial mins over this core's 8 m-tiles; host finishes the min
    colmin_d = nc.dram_tensor("colmin", [P, B], FP16, kind="ExternalOutput")

    with tile.TileContext(nc) as tc:
        with (
            tc.tile_pool(name="consts", bufs=1) as consts,
            tc.tile_pool(name="ytp", bufs=2) as ytp,
            tc.tile_pool(name="xyp", bufs=4) as xyp,
            tc.tile_pool(name="colp", bufs=2) as colp,
            tc.tile_pool(name="pmm", bufs=4, space=bass.MemorySpace.PSUM) as pmm,
        ):
            sqx = consts.tile([P, MT], FP32)
            nc.sync.dma_start(out=sqx[:], in_=sqx_d[:, :])
            rowsl = consts.tile([P, MT], FP32)
            rowacc = consts.tile([P, MT * CHUNK], FP16)
            XT = [
                consts.tile([P, 2, RPC], FP8, tag=f"xt{kg}", name=f"xt{kg}")
                for kg in range(KG)
            ]
            for kg in range(KG):
                nc.sync.dma_start(out=XT[kg][:], in_=xt_d[kg][:, :, :])

            for ch in range(NCH):
                j0 = ch * CHUNK
                yts = []
                for kg in range(KG):
                    ytile = ytp.tile(
                        [P, 2, CHUNK], FP8, tag=f"yt{kg}", name=f"yt{kg}"
                    )
                    for s in range(2):
                        nc.sync.dma_start(
                            out=ytile[:, s, :],
                            in_=yt_d[kg][:, s, j0 : j0 + CHUNK],
                        )
                    yts.append(ytile)

                colmin = colp.tile([P, CHUNK], FP16, tag="colmin")
                for m in range(MT):
                    ms = slice(m * CHUNK, (m + 1) * CHUNK)
                    ps2 = pmm.tile([P, CHUNK], FP32, tag="ps")
                    for h in range(2):
                        hs = slice(h * HALF, (h + 1) * HALF)
                        for kg in range(KG):
                            nc.tensor.matmul(
                                ps2[:, hs],
                                XT[kg][:, :, ts(m, P)],
                                yts[kg][:, :, hs],
                                start=(kg == 0),
                                stop=(kg == KG - 1),
                                perf_mode=mybir.MatmulPerfMode.DoubleRow,
                            )
                    # Act: fp16(-2*psum + sqx_c[m]) over both banks at once
                    mse = xyp.tile([P, CHUNK], FP16, tag="mse")
                    nc.scalar.activation(
                        mse[:], ps2[:, :], AF.Identity,
                        bias=sqx[:, m : m + 1], scale=-2.0,
                    )
                    # DVE: row accumulator (min across chunks, per m)
                    if ch == 0:
                        nc.vector.tensor_copy(rowacc[:, ms], mse[:])
                    else:
                        nc.vector.tensor_tensor(
                            rowacc[:, ms], rowacc[:, ms], mse[:], AL.min
                        )
                    # DVE: col-min accumulator (min across m, per chunk)
                    if m == 0:
                        nc.vector.tensor_copy(colmin[:, :], mse[:])
                    else:
                        nc.vector.tensor_tensor(
                            colmin[:, :], colmin[:, :], mse[:], AL.min
                        )
                    if ch == NCH - 1:
                        nc.vector.tensor_reduce(
                            out=rowsl[:, m : m + 1], in_=rowacc[:, ms],
                            axis=AX.X, op=AL.min,
                        )

                nc.sync.dma_start(
                    out=colmin_d[:, j0 : j0 + CHUNK], in_=colmin[:, :]
                )

            nc.sync.dma_start(out=rowmin_d[:, :], in_=rowsl[:, :])
    if legalize:
        _legalize_waits(nc)
    return nc


_NC_CACHE = None


def _get_nc():
    global _NC_CACHE
    if _NC_CACHE is None:
        _NC_CACHE = build_bass()
    return _NC_CACHE


def _dr_layout(t_km: np.ndarray) -> list[np.ndarray]:
    """[D, cols] contraction-major -> KG DoubleRow tiles [128, 2, cols] where
    tile[kg][p, s, c] = t_km[kg*256 + s*128 + p, c]."""
    d, cols = t_km.shape
    r = t_km.reshape(KG, 2, P, cols).transpose(0, 2, 1, 3)
    return [np.ascontiguousarray(r[kg]) for kg in range(KG)]


def _prep_inputs(X, Y):
    """Host-side sharding/layout: fp8 DoubleRow operands with the last two
    contraction slots repurposed to inject -0.5*(sqy-1024) (hi/lo fp8 pair
    against x-side ones), plus centered fp32 sqx rows. Pure layout/dtype
    prep."""
    sqy_c = ((Y.astype(np.float64) ** 2).sum(axis=1) - float(D)).astype(np.float32)
    t = -0.5 * sqy_c
    t_hi = np.clip(t, -224.0, 224.0).astype(NP_FP8)
    t_lo = (t - t_hi.astype(np.float32)).astype(NP_FP8)
    yt_km = np.empty((D, B), dtype=NP_FP8)
    yt_km[:DF] = Y.T[:DF].astype(NP_FP8)
    yt_km[DF] = t_hi
    yt_km[DF + 1] = t_lo
    yq = _dr_layout(yt_km)

    in_maps = []
    for c in range(NCORES):
        Xs = X[c * RPC : (c + 1) * RPC]
        xt_km = np.empty((D, RPC), dtype=NP_FP8)
        xt_km[:DF] = Xs.T[:DF].astype(NP_FP8)
        xt_km[DF:] = np.float32(1.0)
        xq = _dr_layout(xt_km)
        sqx_c = ((Xs.astype(np.float64) ** 2).sum(axis=1) - float(D)).astype(
            np.float32
        )
        sqx_pm = np.ascontiguousarray(sqx_c.reshape(MT, P).T)
        m = {f"xt{kg}": xq[kg] for kg in range(KG)}
        m.update({f"yt{kg}": yq[kg] for kg in range(KG)})
        m.update({"sqx": sqx_pm})
        in_maps.append(m)
    return in_maps


def kernel(input, target):
    X = np.ascontiguousarray(np.asarray(input, dtype=np.float32))
    Y = np.ascontiguousarray(np.asarray(target, dtype=np.float32))
    assert X.shape == (B, D) and Y.shape == (B, D)

    nc = _get_nc()
    in_maps = _prep_inputs(X, Y)
    try:
        res = run_bass_kernel_spmd(nc, in_maps, core_ids=list(range(NCORES))).results
    except Exception:
        # a prior process can leave a core wedged; one retry clears it
        res = run_bass_kernel_spmd(nc, in_maps, core_ids=list(range(NCORES))).results

    off = np.float64(2.0 * D)
    row_sum = np.float64(0.0)
    col_parts = []
    for r in res:
        row_sum += (r["rowmin"].astype(np.float64) + off).sum()
        col_parts.append(r["colmin"].astype(np.float32).min(axis=0))
    col_min = np.min(np.stack(col_parts), axis=0).astype(np.float64) + off
    loss = (row_sum + col_min.sum()) / D / (2 * B)
    return np.asarray(loss, dtype=np.float32)


# revision 5
# speedup vs baseline: 1.9417x; 1.0031x over previous
"""BatchMatchedMSELoss on 8 Trainium2 NeuronCores.

loss = mean(concat(row_min, col_min)) of the (B,B) pairwise-MSE matrix
  mse[i,j] = (||x_i||^2 + ||y_j||^2 - 2 x_i.y_j) / D,  B=8192, D=1024.

Sharding: input rows split across 8 cores (1024 rows each); every core
computes its (1024, 8192) tile of the centered matrix
  cmse[i,j] = D*mse[i,j] - 2048 = (sqx_i-1024) + (sqy_j-1024) - 2 x_i.y_j
via fp8(e4m3) DoubleRow matmuls (K=256/instruction, 2x bf16 TensorE
throughput). Two of the 1024 contraction slots are donated to carry
-0.5*(sqy-1024) as an fp8 hi/lo pair (x-side slots = 1.0), so PSUM already
holds x.y_1022 - 0.5*sqy_c and no per-column vector add is needed later.
The epilogue splits across the remaining engines to hide behind the
matmul stream:
  * Act : evicts two PSUM banks per op as fp16(-2*psum + sqx_c[m])
          (scale/bias fused; per-partition bias AP) -> full cmse tile
  * DVE : two fast fp16 tensor_tensor(min) accumulations per tile
          (row accumulator per m, col-min per chunk) + one small
          tensor_reduce per m at the end
Host adds back the exact 2048 offset, finishes cross-core/partition
mins and the final mean in fp64. Simulated rel err vs fp32 reference:
2.6e-4 (tolerance 2e-2).
"""

import numpy as np
import ml_dtypes

import concourse.bass as bass
import concourse.tile as tile
import concourse.mybir as mybir
from concourse.bass import ts
from concourse.bass_utils import run_bass_kernel_spmd

FP32 = mybir.dt.float32
FP16 = mybir.dt.float16
FP8 = mybir.dt.float8e4
AL = mybir.AluOpType
AF = mybir.ActivationFunctionType
AX = mybir.AxisListType

B = 8192          # batch (rows of input and target)
D = 1024          # feature dim (contraction); last 2 slots carry sqy hi/lo
DF = D - 2        # real features used in the fp8 cross product
NCORES = 8
RPC = B // NCORES  # rows per core = 1024
P = 128
MT = RPC // P      # 8 row tiles per core
KG = 4             # DoubleRow k-groups (256 contraction rows each)
CHUNK = 1024       # column chunk = one PSUM double-bank eviction
NCH = B // CHUNK   # 8 chunks
HALF = 512         # max moving free dim per matmul / one PSUM bank

NP_FP8 = ml_dtypes.float8_e4m3


def _legalize_waits(nc, max_waits=1):
    """walrus codegen in this container rejects instructions carrying more
    than one sync-wait command. Split extra waits onto standalone
    EventSemaphore instructions (same engine, immediately before), which is
    exactly what engine.wait_ge() emits."""
    n = 0
    for f in nc.m.functions:
        for bb in f.blocks:
            insts = bb.instructions
            out = []
            for inst in insts:
                si = inst.sync_info
                if si is not None and si.on_wait and len(si.on_wait) > max_waits:
                    waits = list(si.on_wait)
                    extra, keep = waits[:-max_waits], waits[-max_waits:]
                    for w in extra:
                        n += 1
                        ev = mybir.InstEventSemaphore(
                            name=f"legwait-{n}-{inst.name}", ins=[], outs=[]
                        )
                        ev.engine = inst.engine
                        ev.sync_info = mybir.SyncInfo(on_wait=[w], on_update=[])
                        out.append(ev)
                    inst.sync_info = mybir.SyncInfo(
                        on_wait=keep, on_update=list(si.on_update)
                    )
                out.append(inst)
            bb.instructions = out
    return n


def build_bass(legalize: bool = True) -> bass.Bass:
    nc = bass.Bass()
    # fp8 operands, contraction-major, DoubleRow layout [128, 2, cols]:
    # element [p, s, c] holds contraction row k = kg*256 + s*128 + p.
    xt_d = [
        nc.dram_tensor(f"xt{kg}", [P, 2, RPC], FP8, kind="ExternalInput")
        for kg in range(KG)
    ]
    yt_d = [
        nc.dram_tensor(f"yt{kg}", [P, 2, B], FP8, kind="ExternalInput")
        for kg in range(KG)
    ]
    # centered fp32 row sq-norms: sqx[p, m] = |x_{m*128+p}|^2 - 1024
    sqx_d = nc.dram_tensor("sqx", [P, MT], FP32, kind="ExternalInput")
    # per-(p, m) partial row minima (host reduces the remaining 1024 cols)
    rowmin_d = nc.dram_tensor("rowmin", [P, MT * CHUNK], FP16, kind="ExternalOutput")
    # column partial mins over this core's 8 m-tiles; host finishes the min
    colmin_d = nc.dram_tensor("colmin", [P, B], FP16, kind="ExternalOutput")

    with tile.TileContext(nc) as tc:
        with (
            tc.tile_pool(name="consts", bufs=1) as consts,
            tc.tile_pool(name="ytp", bufs=2) as ytp,
            tc.tile_pool(name="xyp", bufs=4) as xyp,
            tc.tile_pool(name="colp", bufs=2) as colp,
            tc.tile_pool(name="pmm", bufs=4, space=bass.MemorySpace.PSUM) as pmm,
        ):
            sqx = consts.tile([P, MT], FP32)
            nc.sync.dma_start(out=sqx[:], in_=sqx_d[:, :])
            rowacc = consts.tile([P, MT * CHUNK], FP16)
            XT = [
                consts.tile([P, 2, RPC], FP8, tag=f"xt{kg}", name=f"xt{kg}")
                for kg in range(KG)
            ]

            def load_yts(ch):
                j0 = ch * CHUNK
                yts = []
                for kg in range(KG):
                    ytile = ytp.tile(
                        [P, 2, CHUNK], FP8, tag=f"yt{kg}", name=f"yt{kg}"
                    )
                    # interleave the one-time X loads with chunk 0's Y loads
                    # so the first matmul's inputs land first
                    if ch == 0:
                        nc.sync.dma_start(out=XT[kg][:], in_=xt_d[kg][:, :, :])
                    for s in range(2):
                        nc.sync.dma_start(
                            out=ytile[:, s, :],
                            in_=yt_d[kg][:, s, j0 : j0 + CHUNK],
                        )
                    yts.append(ytile)
                return yts

            for ch in range(NCH):
                j0 = ch * CHUNK
                yts = load_yts(ch)
                colmin = colp.tile([P, CHUNK], FP16, tag="colmin")
                for m in range(MT):
                    ms = slice(m * CHUNK, (m + 1) * CHUNK)
                    ps2 = pmm.tile([P, CHUNK], FP32, tag="ps")
                    for h in range(2):
                        hs = slice(h * HALF, (h + 1) * HALF)
                        for kg in range(KG):
                            nc.tensor.matmul(
                                ps2[:, hs],
                                XT[kg][:, :, ts(m, P)],
                                yts[kg][:, :, hs],
                                start=(kg == 0),
                                stop=(kg == KG - 1),
                                perf_mode=mybir.MatmulPerfMode.DoubleRow,
                            )
                    # Act evicts both banks at once: fp16(-2*psum + sqx_c[m]).
                    # The first chunk / first m write straight into the
                    # accumulators, saving a DVE init pass for each.
                    if ch == 0:
                        tgt = rowacc[:, ms]
                    elif m == 0:
                        tgt = colmin[:, :]
                    else:
                        mse = xyp.tile([P, CHUNK], FP16, tag="mse")
                        tgt = mse[:]
                    nc.scalar.activation(
                        tgt, ps2[:, :], AF.Identity,
                        bias=sqx[:, m : m + 1], scale=-2.0,
                    )
                    # DVE: row accumulator (min across chunks, per m)
                    if ch > 0:
                        nc.vector.tensor_tensor(
                            rowacc[:, ms], rowacc[:, ms], tgt, AL.min
                        )
                    # DVE: col-min accumulator (min across m, per chunk)
                    if ch == 0 and m == 0:
                        nc.vector.tensor_copy(colmin[:, :], tgt)
                    elif m > 0:
                        nc.vector.tensor_tensor(
                            colmin[:, :], colmin[:, :], tgt, AL.min
                        )
                    if ch == NCH - 1:
                        nc.sync.dma_start(
                            out=rowmin_d[:, ms], in_=rowacc[:, ms]
                        )

                nc.sync.dma_start(
                    out=colmin_d[:, j0 : j0 + CHUNK], in_=colmin[:, :]
                )
    if legalize:
        _legalize_waits(nc)
    return nc


_NC_CACHE = None


def _get_nc():
    global _NC_CACHE
    if _NC_CACHE is None:
        _NC_CACHE = build_bass()
    return _NC_CACHE


def _dr_layout(t_km: np.ndarray) -> list[np.ndarray]:
    """[D, cols] contraction-major -> KG DoubleRow tiles [128, 2, cols] where
    tile[kg][p, s, c] = t_km[kg*256 + s*128 + p, c]."""
    d, cols = t_km.shape
    r = t_km.reshape(KG, 2, P, cols).transpose(0, 2, 1, 3)
    return [np.ascontiguousarray(r[kg]) for kg in range(KG)]


def _prep_inputs(X, Y):
    """Host-side sharding/layout: fp8 DoubleRow operands with the last two
    contraction slots repurposed to inject -0.5*(sqy-1024) (hi/lo fp8 pair
    against x-side ones), plus centered fp32 sqx rows. Pure layout/dtype
    prep."""
    sqy_c = ((Y.astype(np.float64) ** 2).sum(axis=1) - float(D)).astype(np.float32)
    t = -0.5 * sqy_c
    t_hi = np.clip(t, -224.0, 224.0).astype(NP_FP8)
    t_lo = (t - t_hi.astype(np.float32)).astype(NP_FP8)
    yt_km = np.empty((D, B), dtype=NP_FP8)
    yt_km[:DF] = Y.T[:DF].astype(NP_FP8)
    yt_km[DF] = t_hi
    yt_km[DF + 1] = t_lo
    yq = _dr_layout(yt_km)

    in_maps = []
    for c in range(NCORES):
        Xs = X[c * RPC : (c + 1) * RPC]
        xt_km = np.empty((D, RPC), dtype=NP_FP8)
        xt_km[:DF] = Xs.T[:DF].astype(NP_FP8)
        xt_km[DF:] = np.float32(1.0)
        xq = _dr_layout(xt_km)
        sqx_c = ((Xs.astype(np.float64) ** 2).sum(axis=1) - float(D)).astype(
            np.float32
        )
        sqx_pm = np.ascontiguousarray(sqx_c.reshape(MT, P).T)
        m = {f"xt{kg}": xq[kg] for kg in range(KG)}
        m.update({f"yt{kg}": yq[kg] for kg in range(KG)})
        m.update({"sqx": sqx_pm})
        in_maps.append(m)
    return in_maps


def kernel(input, target):
    X = np.ascontiguousarray(np.asarray(input, dtype=np.float32))
    Y = np.ascontiguousarray(np.asarray(target, dtype=np.float32))
    assert X.shape == (B, D) and Y.shape == (B, D)

    nc = _get_nc()
    in_maps = _prep_inputs(X, Y)
    try:
        res = run_bass_kernel_spmd(nc, in_maps, core_ids=list(range(NCORES))).results
    except Exception:
        # a prior process can leave a core wedged; one retry clears it
        res = run_bass_kernel_spmd(nc, in_maps, core_ids=list(range(NCORES))).results

    off = np.float64(2.0 * D)
    row_sum = np.float64(0.0)
    col_parts = []
    for r in res:
        rm = r["rowmin"].reshape(P, MT, CHUNK).astype(np.float32).min(axis=2)
        row_sum += (rm.astype(np.float64) + off).sum()
        col_parts.append(r["colmin"].astype(np.float32).min(axis=0))
    col_min = np.min(np.stack(col_parts), axis=0).astype(np.float64) + off
    loss = (row_sum + col_min.sum()) / D / (2 * B)
    return np.asarray(loss, dtype=np.float32)


# revision 8
# speedup vs baseline: 2.0118x; 1.0361x over previous
"""BatchMatchedMSELoss on 8 Trainium2 NeuronCores.

loss = mean(concat(row_min, col_min)) of the (B,B) pairwise-MSE matrix
  mse[i,j] = (||x_i||^2 + ||y_j||^2 - 2 x_i.y_j) / D,  B=8192, D=1024.

Sharding: input rows split across 8 cores (1024 rows each); every core
computes its (1024, 8192) tile of the centered matrix
  cmse[i,j] = D*mse[i,j] - 2048 = (sqx_i-1024) + (sqy_j-1024) - 2 x_i.y_j
via fp8(e4m3) DoubleRow matmuls (K=256/instruction, 2x bf16 TensorE
throughput). Two of the 1024 contraction slots are donated to carry
-0.5*(sqy-1024) as an fp8 hi/lo pair (x-side slots = 1.0), so PSUM already
holds x.y_1022 - 0.5*sqy_c and no per-column vector add is needed later.
The epilogue splits across the remaining engines to hide behind the
matmul stream:
  * Act : evicts two PSUM banks per op as fp16(-2*psum + sqx_c[m])
          (scale/bias fused; per-partition bias AP) -> full cmse tile
  * DVE : two fast fp16 tensor_tensor(min) accumulations per tile
          (row accumulator per m, col-min per chunk) + one small
          tensor_reduce per m at the end
Host adds back the exact 2048 offset, finishes cross-core/partition
mins and the final mean in fp64. Simulated rel err vs fp32 reference:
2.6e-4 (tolerance 2e-2).
"""

import numpy as np
import ml_dtypes

import concourse.bass as bass
import concourse.tile as tile
import concourse.mybir as mybir
from concourse.bass import ts
from concourse.bass_utils import run_bass_kernel_spmd

FP32 = mybir.dt.float32
FP16 = mybir.dt.float16
FP8 = mybir.dt.float8e4
AL = mybir.AluOpType
AF = mybir.ActivationFunctionType
AX = mybir.AxisListType

B = 8192          # batch (rows of input and target)
D = 1024          # feature dim (contraction); last 2 slots carry sqy hi/lo
DF = D - 2        # real features used in the fp8 cross product
NCORES = 8
RPC = B // NCORES  # rows per core = 1024
P = 128
MT = RPC // P      # 8 row tiles per core
KG = 4             # DoubleRow k-groups (256 contraction rows each)
CHUNK = 1024       # column chunk = one PSUM double-bank eviction
NCH = B // CHUNK   # 8 chunks
HALF = 512         # max moving free dim per matmul / one PSUM bank

NP_FP8 = ml_dtypes.float8_e4m3


def _legalize_waits(nc, max_waits=1):
    """walrus codegen in this container rejects instructions carrying more
    than one sync-wait command. Split extra waits onto standalone
    EventSemaphore instructions (same engine, immediately before), which is
    exactly what engine.wait_ge() emits."""
    n = 0
    for f in nc.m.functions:
        for bb in f.blocks:
            insts = bb.instructions
            out = []
            for inst in insts:
                si = inst.sync_info
                if si is not None and si.on_wait and len(si.on_wait) > max_waits:
                    waits = list(si.on_wait)
                    extra, keep = waits[:-max_waits], waits[-max_waits:]
                    for w in extra:
                        n += 1
                        ev = mybir.InstEventSemaphore(
                            name=f"legwait-{n}-{inst.name}", ins=[], outs=[]
                        )
                        ev.engine = inst.engine
                        ev.sync_info = mybir.SyncInfo(on_wait=[w], on_update=[])
                        out.append(ev)
                    inst.sync_info = mybir.SyncInfo(
                        on_wait=keep, on_update=list(si.on_update)
                    )
                out.append(inst)
            bb.instructions = out
    return n


def build_bass(legalize: bool = True) -> bass.Bass:
    nc = bass.Bass()
    # fp8 operands, contraction-major, DoubleRow layout [128, 2, cols]:
    # element [p, s, c] holds contraction row k = kg*256 + s*128 + p.
    xt_d = [
        nc.dram_tensor(f"xt{kg}", [P, 2, RPC], FP8, kind="ExternalInput")
        for kg in range(KG)
    ]
    yt_d = [
        nc.dram_tensor(f"yt{kg}", [P, 2, B], FP8, kind="ExternalInput")
        for kg in range(KG)
    ]
    # centered fp32 row sq-norms: sqx[p, m] = |x_{m*128+p}|^2 - 1024
    sqx_d = nc.dram_tensor("sqx", [P, MT], FP32, kind="ExternalInput")
    # per-(p, m) partial row minima (host reduces the remaining 1024 cols)
    rowmin_d = nc.dram_tensor("rowmin", [P, MT * CHUNK], FP16, kind="ExternalOutput")
    # column partial mins over this core's 8 m-tiles; host finishes the min
    colmin_d = nc.dram_tensor("colmin", [P, B], FP16, kind="ExternalOutput")

    with tile.TileContext(nc) as tc:
        with (
            tc.tile_pool(name="consts", bufs=1) as consts,
            tc.tile_pool(name="ytp", bufs=2) as ytp,
            tc.tile_pool(name="xyp", bufs=4) as xyp,
            tc.tile_pool(name="colp", bufs=2) as colp,
            tc.tile_pool(name="pmm", bufs=4, space=bass.MemorySpace.PSUM) as pmm,
        ):
            sqx = consts.tile([P, MT], FP32)
            rowacc = consts.tile([P, MT * CHUNK], FP16)
            XT = [
                consts.tile([P, 2, RPC], FP8, tag=f"xt{kg}", name=f"xt{kg}")
                for kg in range(KG)
            ]
            # X-side loads ride the Act engine's DGE queue so they dispatch
            # in parallel with the Y loads on the sync queue
            for kg in range(KG):
                nc.scalar.dma_start(out=XT[kg][:], in_=xt_d[kg][:, :, :])
            nc.scalar.dma_start(out=sqx[:], in_=sqx_d[:, :])

            def load_yts(ch):
                j0 = ch * CHUNK
                yts = []
                for kg in range(KG):
                    ytile = ytp.tile(
                        [P, 2, CHUNK], FP8, tag=f"yt{kg}", name=f"yt{kg}"
                    )
                    for s in range(2):
                        nc.sync.dma_start(
                            out=ytile[:, s, :],
                            in_=yt_d[kg][:, s, j0 : j0 + CHUNK],
                        )
                    yts.append(ytile)
                return yts

            for ch in range(NCH):
                j0 = ch * CHUNK
                yts = load_yts(ch)
                colmin = colp.tile([P, CHUNK], FP16, tag="colmin")
                last_ch = ch == NCH - 1
                for m in range(MT):
                    ms = slice(m * CHUNK, (m + 1) * CHUNK)
                    last = last_ch and m == MT - 1
                    ps2 = pmm.tile([P, CHUNK], FP32, tag="ps")
                    if last:
                        mse7 = xyp.tile([P, CHUNK], FP16, tag="mse")
                    for h in range(2):
                        hs = slice(h * HALF, (h + 1) * HALF)
                        for kg in range(KG):
                            nc.tensor.matmul(
                                ps2[:, hs],
                                XT[kg][:, :, ts(m, P)],
                                yts[kg][:, :, hs],
                                start=(kg == 0),
                                stop=(kg == KG - 1),
                                perf_mode=mybir.MatmulPerfMode.DoubleRow,
                            )
                        if last:
                            # drain the very last tile at half granularity so
                            # the epilogue chain overlaps the final matmuls
                            nc.scalar.activation(
                                mse7[:, hs], ps2[:, hs], AF.Identity,
                                bias=sqx[:, m : m + 1], scale=-2.0,
                            )
                            nc.vector.tensor_tensor(
                                colmin[:, hs], colmin[:, hs], mse7[:, hs], AL.min
                            )
                            rs = slice(m * CHUNK + h * HALF, m * CHUNK + (h + 1) * HALF)
                            nc.vector.tensor_tensor(
                                rowacc[:, rs], rowacc[:, rs], mse7[:, hs], AL.min
                            )
                            nc.sync.dma_start(
                                out=colmin_d[:, j0 + h * HALF : j0 + (h + 1) * HALF],
                                in_=colmin[:, hs],
                            )
                            nc.sync.dma_start(
                                out=rowmin_d[:, rs], in_=rowacc[:, rs]
                            )
                    if last:
                        continue
                    # Act evicts both banks at once: fp16(-2*psum + sqx_c[m]).
                    # The first chunk / first m write straight into the
                    # accumulators, saving a DVE init pass for each.
                    if ch == 0:
                        tgt = rowacc[:, ms]
                    elif m == 0:
                        tgt = colmin[:, :]
                    else:
                        mse = xyp.tile([P, CHUNK], FP16, tag="mse")
                        tgt = mse[:]
                    nc.scalar.activation(
                        tgt, ps2[:, :], AF.Identity,
                        bias=sqx[:, m : m + 1], scale=-2.0,
                    )
                    # DVE: row accumulator (min across chunks, per m)
                    if ch > 0:
                        nc.vector.tensor_tensor(
                            rowacc[:, ms], rowacc[:, ms], tgt, AL.min
                        )
                    # DVE: col-min accumulator (min across m, per chunk)
                    if ch == 0 and m == 0:
                        nc.vector.tensor_copy(colmin[:, :], tgt)
                    elif m > 0:
                        nc.vector.tensor_tensor(
                            colmin[:, :], colmin[:, :], tgt, AL.min
                        )
                    if last_ch:
                        nc.sync.dma_start(
                            out=rowmin_d[:, ms], in_=rowacc[:, ms]
                        )

                if not last_ch:
                    nc.sync.dma_start(
                        out=colmin_d[:, j0 : j0 + CHUNK], in_=colmin[:, :]
                    )
    if legalize:
        _legalize_waits(nc)
    return nc


_NC_CACHE = None


def _get_nc():
    global _NC_CACHE
    if _NC_CACHE is None:
        _NC_CACHE = build_bass()
    return _NC_CACHE


def _dr_layout(t_km: np.ndarray) -> list[np.ndarray]:
    """[D, cols] contraction-major -> KG DoubleRow tiles [128, 2, cols] where
    tile[kg][p, s, c] = t_km[kg*256 + s*128 + p, c]."""
    d, cols = t_km.shape
    r = t_km.reshape(KG, 2, P, cols).transpose(0, 2, 1, 3)
    return [np.ascontiguousarray(r[kg]) for kg in range(KG)]


def _prep_inputs(X, Y):
    """Host-side sharding/layout: fp8 DoubleRow operands with the last two
    contraction slots repurposed to inject -0.5*(sqy-1024) (hi/lo fp8 pair
    against x-side ones), plus centered fp32 sqx rows. Pure layout/dtype
    prep."""
    sqy_c = ((Y.astype(np.float64) ** 2).sum(axis=1) - float(D)).astype(np.float32)
    t = -0.5 * sqy_c
    t_hi = np.clip(t, -224.0, 224.0).astype(NP_FP8)
    t_lo = (t - t_hi.astype(np.float32)).astype(NP_FP8)
    yt_km = np.empty((D, B), dtype=NP_FP8)
    yt_km[:DF] = Y.T[:DF].astype(NP_FP8)
    yt_km[DF] = t_hi
    yt_km[DF + 1] = t_lo
    yq = _dr_layout(yt_km)

    in_maps = []
    for c in range(NCORES):
        Xs = X[c * RPC : (c + 1) * RPC]
        xt_km = np.empty((D, RPC), dtype=NP_FP8)
        xt_km[:DF] = Xs.T[:DF].astype(NP_FP8)
        xt_km[DF:] = np.float32(1.0)
        xq = _dr_layout(xt_km)
        sqx_c = ((Xs.astype(np.float64) ** 2).sum(axis=1) - float(D)).astype(
            np.float32
        )
        sqx_pm = np.ascontiguousarray(sqx_c.reshape(MT, P).T)
        m = {f"xt{kg}": xq[kg] for kg in range(KG)}
        m.update({f"yt{kg}": yq[kg] for kg in range(KG)})
        m.update({"sqx": sqx_pm})
        in_maps.append(m)
    return in_maps


def kernel(input, target):
    X = np.ascontiguousarray(np.asarray(input, dtype=np.float32))
    Y = np.ascontiguousarray(np.asarray(target, dtype=np.float32))
    assert X.shape == (B, D) and Y.shape == (B, D)

    nc = _get_nc()
    in_maps = _prep_inputs(X, Y)
    try:
        res = run_bass_kernel_spmd(nc, in_maps, core_ids=list(range(NCORES))).results
    except Exception:
        # a prior process can leave a core wedged; one retry clears it
        res = run_bass_kernel_spmd(nc, in_maps, core_ids=list(range(NCORES))).results

    off = np.float64(2.0 * D)
    row_sum = np.float64(0.0)
    col_parts = []
    for r in res:
        rm = r["rowmin"].reshape(P, MT, CHUNK).astype(np.float32).min(axis=2)
        row_sum += (rm.astype(np.float64) + off).sum()
        col_parts.append(r["colmin"].astype(np.float32).min(axis=0))
    col_min = np.min(np.stack(col_parts), axis=0).astype(np.float64) + off
    loss = (row_sum + col_min.sum()) / D / (2 * B)
    return np.asarray(loss, dtype=np.float32)
